# revision 3
# baseline (speedup 1.0000x reference)
"""GCN (5-layer PyG GCNConv + BatchNorm eval + ReLU) on 8 Trainium2 NeuronCores.

v2 design. Nodes are dst-sharded across 8 cores (12544 padded rows each,
98 dst tiles of 128); edges follow their destination. Per layer, each core
computes hd = (act @ W')*dinv in bf16 for its own nodes; node rows are split
into 4 quarters, and each quarter is AllGathered (bf16) as soon as its tiles
finish, overlapping collectives with compute. Aggregation runs per group of
dst tiles: one merged dma_gather per (group, quarter) pulls all edge source
rows (256B bf16) from the quarter's gathered table; one-hot selection
matrices are built in a single broadcast is_equal per (group, quarter) and
used as bf16 stationary operands in PSUM-accumulating matmuls. The self-loop
term is accumulated with an identity matmul from the SBUF-resident hd, and
the (BN-folded) bias with a rank-1 outer-product matmul of sqrt(deg) x bias,
so the epilogue is a single fused scale+ReLU activation. Layer 5 applies W5
before aggregation (width 2). All edge bookkeeping (gather indices, one-hot
slot tables, paddings) is precomputed on the host from edge_index.

GCN_REPS=n unrolls the whole computation n times inside the device program
(identical results each iteration) so test.py can measure device execution
time as a wall-clock slope, independent of the host<->device tunnel latency.
"""
import os
import numpy as np
import ml_dtypes

N = 100000
E = 1600000
IN = 128
H = 128
C = 2
EPS = 1e-5
NC = 8
SR = 12500            # real nodes per core
P = 128
TP = 98               # dst tiles per core
SH = TP * P           # padded nodes per core = 12544
NQ = 4                # quarters (gather chunks == collective pieces)
QT = [25, 25, 24, 24]             # tiles per quarter
QTOFF = [0, 25, 50, 74]
QROFF = [0, 3200, 6400, 9472]     # row offsets per core
QROWS = [3200, 3200, 3072, 3072]  # rows per quarter per core
GROUPS = [5] * 19 + [3]           # dst tiles per aggregation group
NG = len(GROUPS)
GOFF = np.cumsum([0] + GROUPS).tolist()
DIMS = [(IN, H), (H, H), (H, H), (H, H // 2), (H // 2, C)]
AGG_D = [128, 128, 128, 64, 2]

_cache = {}

# ---------------------------------------------------------------------------
# Tile patch: walrus in this container rejects TPB_CTRL/extended instructions
# with >1 sync wait. Split waits across single-wait NOPs.
# ---------------------------------------------------------------------------


def _apply_tile_patch():
    if _cache.get("patched"):
        return
    _cache["patched"] = True
    import concourse.tile as tile_mod
    import concourse.mybir as mybir
    from concourse.vector_clock import ScopedClock

    MAXW = 1

    def _patched_drain_and_barrier(self, tick_clock, wait_clock):
        nc = self.nc
        probe = nc.sync.nop(nofuse=True)
        wait_clock.add_sem_waits(probe.ins, ScopedClock({None: tick_clock.global_clock}))
        si = probe.ins.sync_info
        if si is not None and si.on_wait and len(si.on_wait) > MAXW:
            waits = list(si.on_wait)
            si.on_wait = waits[:MAXW]
            for k in range(MAXW, len(waits), MAXW):
                extra = nc.sync.nop(nofuse=True)
                esi = extra.ins.sync_info
                if esi is None:
                    extra.ins.sync_info = mybir.SyncInfo(
                        on_wait=waits[k:k + MAXW], on_update=[]
                    )
                else:
                    esi.on_wait = waits[k:k + MAXW]
        nc.sync.drain()
        nc.all_engine_barrier()
        assert self.sems is not None
        popped = nc._tile_sem_poison_stack.pop()
        assert popped is self._sem_poison
        nc.clear_and_free_semaphores(list(self.sems.allocated().values()))
        nc.all_engine_barrier()

    tile_mod.TileContext._drain_and_barrier = _patched_drain_and_barrier

    _orig_commit = tile_mod.TileContext._commit_instruction

    def _patched_commit_instruction(self, inst, lazy_reg_writes=True):
        si = getattr(inst, "sync_info", None)
        if (
            si is not None
            and si.on_wait
            and len(si.on_wait) > MAXW
            and inst.engine != mybir.EngineType.Unassigned
        ):
            waits = list(si.on_wait)
            si.on_wait = waits[:MAXW]
            eng = self.nc.engines[inst.engine]
            for k in range(MAXW, len(waits), MAXW):
                extra = eng.nop(nofuse=True)
                esi = extra.ins.sync_info
                chunk = waits[k:k + MAXW]
                if esi is None:
                    extra.ins.sync_info = mybir.SyncInfo(on_wait=chunk, on_update=[])
                else:
                    esi.on_wait = chunk
        return _orig_commit(self, inst, lazy_reg_writes)

    tile_mod.TileContext._commit_instruction = _patched_commit_instruction


# ---------------------------------------------------------------------------
# SPMD runner: compile once via bass2jax/PJRT, keep the jitted fn for reuse.
# ---------------------------------------------------------------------------


class _SpmdRunner:
    def __init__(self, nc, n_cores=8):
        import jax
        from jax.sharding import Mesh, PartitionSpec, NamedSharding
        from jax.experimental.shard_map import shard_map
        import concourse.mybir as mybir
        from concourse.bass2jax import (
            _bass_exec_p,
            install_neuronx_cc_hook,
            partition_id_tensor,
        )
        from concourse.library_overlay import lower_extended_insts

        lower_extended_insts(nc)
        install_neuronx_cc_hook()
        self.jax = jax
        self.n_cores = n_cores
        partition_name = nc.partition_id_tensor.name if nc.partition_id_tensor else None
        in_names, out_names, out_avals, zero_outs = [], [], [], []
        for alloc in nc.m.functions[0].allocations:
            if not isinstance(alloc, mybir.MemoryLocationSet):
                continue
            name = alloc.memorylocations[0].name
            if alloc.kind == "ExternalInput":
                if name != partition_name:
                    in_names.append(name)
            elif alloc.kind == "ExternalOutput":
                out_names.append(name)
                shape = tuple(alloc.tensor_shape)
                dtype = mybir.dt.np(alloc.dtype)
                out_avals.append(jax.core.ShapedArray(shape, dtype))
                zero_outs.append(np.zeros(shape, dtype))
        self.in_names = list(in_names)
        self.out_names = out_names
        self.out_avals = out_avals
        self.zero_outs = zero_outs
        n_params = len(in_names)
        n_outs = len(out_avals)
        all_in_names = list(in_names) + list(out_names)
        if partition_name is not None:
            all_in_names.append(partition_name)

        def _body(*args):
            operands = list(args)
            if partition_name is not None:
                operands.append(partition_id_tensor())
            outs = _bass_exec_p.bind(
                *operands,
                out_avals=tuple(out_avals),
                in_names=tuple(all_in_names),
                out_names=tuple(out_names),
                lowering_input_output_aliases=(),
                sim_require_finite=True,
                sim_require_nnan=True,
                nc=nc,
            )
            return tuple(outs)

        devices = jax.devices()[:n_cores]
        self.mesh = Mesh(np.asarray(devices), ("core",))
        in_specs = (PartitionSpec("core"),) * (n_params + n_outs)
        out_specs = (PartitionSpec("core"),) * n_outs
        self.sharding = NamedSharding(self.mesh, PartitionSpec("core"))
        self.fn = jax.jit(
            shard_map(
                _body, mesh=self.mesh, in_specs=in_specs, out_specs=out_specs,
                check_rep=False,
            ),
            keep_unused=True,
        )
        self.n_params = n_params

    def put_inputs(self, in_maps):
        jax = self.jax
        per_core = [[np.asarray(m[name]) for name in self.in_names] for m in in_maps]
        concat_in = [
            np.concatenate([per_core[c][i] for c in range(self.n_cores)], axis=0)
            for i in range(self.n_params)
        ]
        self.dev_in = [jax.device_put(a, self.sharding) for a in concat_in]
        self.dev_zeros = [
            jax.device_put(
                np.zeros((self.n_cores * z.shape[0], *z.shape[1:]), z.dtype),
                self.sharding,
            )
            for z in self.zero_outs
        ]
        jax.block_until_ready(self.dev_in)

    def run(self):
        outs = self.fn(*self.dev_in, *self.dev_zeros)
        self.jax.block_until_ready(outs)
        return outs

    def results(self, outs):
        res = []
        for c in range(self.n_cores):
            res.append(
                {
                    name: np.asarray(outs[i]).reshape(
                        self.n_cores, *self.out_avals[i].shape
                    )[c]
                    for i, name in enumerate(self.out_names)
                }
            )
        return res

    def time_runs(self, n=6):
        import time
        ts = []
        for _ in range(n):
            t0 = time.perf_counter()
            self.run()
            ts.append(time.perf_counter() - t0)
        return ts


# ---------------------------------------------------------------------------
# Host-side graph partitioning
# ---------------------------------------------------------------------------


def _host_prep(edge_index):
    src = np.asarray(edge_index[0], dtype=np.int64)
    dst = np.asarray(edge_index[1], dtype=np.int64)
    deg = np.bincount(dst, minlength=N).astype(np.float32) + 1.0
    dinv = (1.0 / np.sqrt(deg)).astype(np.float32)

    core = dst // SR
    dl = dst - core * SR
    tile = dl // P
    dslot = dl % P
    grp = np.searchsorted(np.asarray(GOFF[1:]), tile, side="right")
    trel = tile - np.asarray(GOFF)[grp]

    cs = src // SR
    r = src - cs * SR
    q = (r >= QROFF[1]).astype(np.int64) + (r >= QROFF[2]) + (r >= QROFF[3])
    qrows = np.asarray(QROWS)[q]
    qidx = cs * qrows + (r - np.asarray(QROFF)[q])

    GMAX = max(GROUPS)
    cell = (((core * NG + grp) * NQ + q) * GMAX + trel).astype(np.int64)
    ncells = NC * NG * NQ * GMAX
    order = np.lexsort((qidx, cell))
    cell_s = cell[order]
    qidx_s = qidx[order]
    dslot_s = dslot[order]

    cnt = np.bincount(cell_s, minlength=ncells)
    cnt4 = cnt.reshape(NC, NG, NQ, GMAX)
    # per (group, quarter) block capacity: max over cores and tiles in group
    bcap = np.zeros((NG, NQ), np.int64)
    for g in range(NG):
        for qq in range(NQ):
            m = cnt4[:, g, qq, :GROUPS[g]].max()
            bcap[g, qq] = max(1, int(np.ceil(m / P)))
    TBg = bcap.sum(axis=1)

    # per-edge slot position
    gstart = np.zeros(ncells + 1, np.int64)
    np.cumsum(cnt, out=gstart[1:])
    rank = np.arange(E) - gstart[cell_s]

    g_s = (cell_s // (NQ * GMAX)) % NG
    q_s = (cell_s // GMAX) % NQ
    t_s = cell_s % GMAX
    c_core = cell_s // (NG * NQ * GMAX)

    # column of edge inside its group's gather buffer
    cumb = np.zeros((NG, NQ + 1), np.int64)
    for g in range(NG):
        cumb[g, 1:] = np.cumsum(bcap[g]) * GROUPS[g]
    colrel = cumb[g_s, q_s] + t_s * bcap[g_s, q_s] + rank // P
    rowrel = rank % P

    # global column offsets of each group's buffers
    WG = [GROUPS[g] * int(TBg[g]) for g in range(NG)]   # columns per group
    WOFF = np.cumsum([0] + WG)
    DW = int(WOFF[-1])

    # dsel table [NC, 128, DW] (f32; -1 pad)
    dsel_tab = np.full((NC, P, DW), -1.0, np.float32)
    dsel_tab[c_core, rowrel, WOFF[g_s] + colrel] = dslot_s.astype(np.float32)

    # idx sequences: per (core, group, quarter) call, flat length G*bcap*128,
    # tile-major, pad 0
    idx_flat = np.zeros((NC, DW, P), np.int16)  # [core, col, row-in-block]
    idx_flat[c_core, WOFF[g_s] + colrel, rowrel] = qidx_s.astype(np.int16)

    # wrap each call's flat sequence into the Q7 16-partition window layout
    IW = DW * 8
    idx_w = np.zeros((NC, P, IW), np.int16)
    for g in range(NG):
        for qq in range(NQ):
            c0 = int(WOFF[g] + cumb[g, qq])
            w = GROUPS[g] * int(bcap[g, qq])          # columns in this call
            seq = idx_flat[:, c0:c0 + w, :].reshape(NC, w * P)
            w16 = seq.reshape(NC, w * P // 16, 16).transpose(0, 2, 1)  # [NC,16,L/16]
            tiled = np.tile(w16, (1, 8, 1))           # [NC, 128, L/16]
            idx_w[:, :, c0 * 8: (c0 + w) * 8] = tiled
    return dinv, idx_w, dsel_tab, bcap, TBg, cumb, WOFF


def _fold_weights(inputs):
    Ws, Bs = [], []
    for i in range(1, 6):
        W = np.asarray(inputs[f"W{i}"], np.float32)
        b = np.asarray(inputs[f"b{i}"], np.float32)
        if i <= 4:
            g = np.asarray(inputs[f"g{i}"], np.float32)
            be = np.asarray(inputs[f"be{i}"], np.float32)
            rm = np.asarray(inputs[f"rm{i}"], np.float32)
            rv = np.asarray(inputs[f"rv{i}"], np.float32)
            s = g / np.sqrt(rv + EPS)
            W = W * s[None, :]
            b = b * s + be - rm * s
        Ws.append(np.ascontiguousarray(W, dtype=np.float32))
        Bs.append(b.astype(np.float32)[None, :])
    return Ws, Bs


# ---------------------------------------------------------------------------
# Device program
# ---------------------------------------------------------------------------


def _build_nc(bcap, TBg, cumb, WOFF):
    NLAY = int(os.environ.get("GCN_LAYERS", 5))
    REPS = int(os.environ.get("GCN_REPS", 1))
    import concourse.bass as bass
    import concourse.mybir as mybir
    from concourse.tile import TileContext
    from concourse import library_config

    _apply_tile_patch()

    f32 = mybir.dt.float32
    bf16 = mybir.dt.bfloat16
    i16 = mybir.dt.int16
    nc = bass.Bass("TRN2", target_bir_lowering=False, debug=False, num_swdge_queues=4)

    DW = int(WOFF[-1])
    IW = DW * 8
    WGMAX = max(GROUPS[g] * int(TBg[g]) for g in range(NG))
    BWMAX = max(
        GROUPS[g] * int(bcap[g][qq]) for g in range(NG) for qq in range(NQ)
    )

    xT_in = nc.declare_dram_parameter("xT", [IN, SH], bf16, isOutput=False)
    dinv_in = nc.declare_dram_parameter("dinv", [P, TP], f32, isOutput=False)
    rdinv_in = nc.declare_dram_parameter("rdinv", [1, SH], bf16, isOutput=False)
    idx_in = nc.declare_dram_parameter("idx", [P, IW], i16, isOutput=False)
    dsel_in = nc.declare_dram_parameter("dsel", [P, DW], f32, isOutput=False)
    iota_in = nc.declare_dram_parameter("iotaw", [P, BWMAX * P], f32, isOutput=False)
    W_in = [nc.declare_dram_parameter(f"W{i+1}", list(DIMS[i]), bf16, isOutput=False) for i in range(5)]
    B_in = [nc.declare_dram_parameter(f"B{i+1}", [1, DIMS[i][1]], bf16, isOutput=False) for i in range(5)]
    ident_in = nc.declare_dram_parameter("ident", [P, P], bf16, isOutput=False)
    y_out = nc.declare_dram_parameter("y", [SH, C], f32, isOutput=True)

    in_b = [
        [nc.dram_tensor(f"in_b{l}_{q}", [QROWS[q], P], bf16) for q in range(NQ)]
        for l in range(5)
    ]
    hs = [
        [
            nc.dram_tensor(f"hs{l}_{q}", [NC * QROWS[q], P], bf16, addr_space="Shared")
            for q in range(NQ)
        ]
        for l in range(5)
    ]

    with TileContext(nc) as tc:
        with (
            tc.tile_pool(name="const", bufs=1) as cpool,
            tc.tile_pool(name="act", bufs=1) as apool,
            tc.tile_pool(name="hbuf", bufs=1) as hpool,
            tc.tile_pool(name="gath", bufs=2) as gpool,
            tc.tile_pool(name="sel", bufs=2) as sbpool,
            tc.tile_pool(name="idxp", bufs=4) as ipool,
            tc.tile_pool(name="dselp", bufs=4) as dpool,
            tc.tile_pool(name="work", bufs=4) as wpool,
            tc.tile_pool(name="ps_h", bufs=2, space="PSUM") as ps_h,
            tc.tile_pool(name="ps_a", bufs=2, space="PSUM") as ps_a,
            tc.tile_pool(name="ps_t", bufs=2, space="PSUM") as ps_t,
        ):
            nc.gpsimd.load_library(library_config.mlp)
            # registers for gather num_idxs (per distinct value)
            nid_vals = sorted({
                GROUPS[g] * int(bcap[g][qq]) * P
                for g in range(NG) for qq in range(NQ)
            })
            nid_regs = {}
            for v in nid_vals:
                reg = nc.alloc_register(mybir.EngineType.Pool, f"nid{v}")
                nc.gpsimd.reg_mov(reg, v)
                nid_regs[v] = reg

            Wt, Bt = [], []
            for l in range(5):
                w = cpool.tile(list(DIMS[l]), bf16, name=f"Wt{l}")
                nc.sync.dma_start(out=w[:], in_=W_in[l][:])
                Wt.append(w)
                b = cpool.tile([1, DIMS[l][1]], bf16, name=f"Bt{l}")
                nc.sync.dma_start(out=b[:], in_=B_in[l][:])
                Bt.append(b)
            iota_t = cpool.tile([P, BWMAX * P], f32)
            nc.sync.dma_start(out=iota_t[:], in_=iota_in[:])
            ident_t = cpool.tile([P, P], bf16)
            nc.sync.dma_start(out=ident_t[:], in_=ident_in[:])
            dinv_t = cpool.tile([P, TP], f32)
            nc.sync.dma_start(out=dinv_t[:], in_=dinv_in[:])
            rdinv_t = cpool.tile([1, SH], bf16)
            nc.sync.dma_start(out=rdinv_t[:], in_=rdinv_in[:])

            for rep in range(REPS):
                actT = apool.tile([P, SH], bf16, tag="actT")
                nc.sync.dma_start(out=actT[:IN, :], in_=xT_in[:])
                h_s = hpool.tile([P, TP * P], bf16, tag="h_s")

                for l in range(NLAY):
                    I, O = DIMS[l]
                    D = AGG_D[l]
                    # ---- H phase: hd = (act @ W')*dinv, quartered AllGather
                    for q in range(NQ):
                        for t in range(QTOFF[q], QTOFF[q] + QT[q]):
                            ps = ps_h.tile([P, O], f32, tag="ps_h")
                            nc.tensor.matmul(
                                out=ps[:], lhsT=actT[:I, t * P:(t + 1) * P],
                                rhs=Wt[l][:], start=True, stop=True,
                            )
                            nc.vector.tensor_scalar_mul(
                                out=h_s[:, t * O:(t + 1) * O], in0=ps[:],
                                scalar1=dinv_t[:, t:t + 1],
                            )
                            r0 = t * P - QROFF[q]
                            nc.sync.dma_start(
                                out=in_b[l][q].ap()[r0:r0 + P, :O],
                                in_=h_s[:, t * O:(t + 1) * O],
                            )
                        nc.gpsimd.collective_compute(
                            "AllGather",
                            mybir.AluOpType.bypass,
                            ins=[in_b[l][q][:]],
                            outs=[hs[l][q][:]],
                            replica_groups=[list(range(NC))],
                        )
                    # ---- aggregation phase, grouped
                    for g in range(NG):
                        Gg = GROUPS[g]
                        WGg = Gg * int(TBg[g])
                        c0g = int(WOFF[g])
                        idx_t = ipool.tile([P, WGMAX * 8], i16, tag="idx")
                        nc.scalar.dma_start(
                            out=idx_t[:, :WGg * 8],
                            in_=idx_in.ap()[:, c0g * 8:(c0g + WGg) * 8],
                        )
                        dsel_t = dpool.tile([P, WGMAX], f32, tag="dsel")
                        nc.scalar.dma_start(
                            out=dsel_t[:, :WGg],
                            in_=dsel_in.ap()[:, c0g:c0g + WGg],
                        )
                        gbuf = gpool.tile([P, WGMAX, P], bf16, tag="g")
                        St = sbpool.tile([P, WGMAX * P], bf16, tag="S")
                        for q in range(NQ):
                            w = Gg * int(bcap[g][q])
                            cq = int(cumb[g][q])
                            nc.gpsimd.dma_gather(
                                out_ap=gbuf[:, cq:cq + w, :],
                                in_ap=hs[l][q].ap()[:, :],
                                idxs_ap=idx_t[:, (cq) * 8:(cq + w) * 8],
                                num_idxs=w * P,
                                num_idxs_reg=nid_regs[w * P],
                                elem_size=P,
                                single_packet=False,
                                queue_num=(g * NQ + q) % 4,
                            )
                            nc.vector.tensor_tensor(
                                out=St[:, cq * P:(cq + w) * P].rearrange(
                                    "p (b q) -> p b q", b=w
                                ),
                                in0=iota_t[:, :w * P].rearrange(
                                    "p (b q) -> p b q", b=w
                                ),
                                in1=dsel_t[:, cq:cq + w].unsqueeze(2).broadcast_to(
                                    [P, w, P]
                                ),
                                op=mybir.AluOpType.is_equal,
                            )
                        for trel in range(Gg):
                            t = GOFF[g] + trel
                            pa = ps_a.tile([P, D], f32, tag="pa")
                            first = True
                            for q in range(NQ):
                                bq = int(bcap[g][q])
                                for j in range(bq):
                                    cc = int(cumb[g][q]) + trel * bq + j
                                    nc.tensor.matmul(
                                        out=pa[:],
                                        lhsT=St[:, cc * P:(cc + 1) * P],
                                        rhs=gbuf[:, cc, :D],
                                        start=first, stop=False,
                                    )
                                    first = False
                            nc.tensor.matmul(
                                out=pa[:], lhsT=ident_t[:],
                                rhs=h_s[:, t * O:(t + 1) * O],
                                start=False, stop=False,
                            )
                            nc.tensor.matmul(
                                out=pa[:],
                                lhsT=rdinv_t[:, t * P:(t + 1) * P],
                                rhs=Bt[l][:],
                                start=False, stop=True,
                            )
                            if l < 4:
                                act_t = wpool.tile([P, O], bf16, tag="actn")
                                nc.scalar.activation(
                                    out=act_t[:], in_=pa[:],
                                    func=mybir.ActivationFunctionType.Relu,
                                    scale=dinv_t[:, t:t + 1],
                                )
                                pt = ps_t.tile([P, P], bf16, tag="pt")
                                nc.tensor.transpose(
                                    out=pt[:O, :], in_=act_t[:], identity=ident_t[:]
                                )
                                nc.vector.tensor_copy(
                                    out=actT[:O, t * P:(t + 1) * P], in_=pt[:O, :]
                                )
                            else:
                                yt = wpool.tile([P, C], f32, tag="yt")
                                nc.vector.tensor_scalar_mul(
                                    out=yt[:], in0=pa[:], scalar1=dinv_t[:, t:t + 1]
                                )
                                nc.sync.dma_start(
                                    out=y_out.ap()[t * P:(t + 1) * P, :], in_=yt[:]
                                )
    return nc


def kernel(**inputs):
    edge_index = np.asarray(inputs["edge_index"])
    key = edge_index.tobytes()[:64]
    if "prep" not in _cache or _cache.get("key") != key:
        _cache["key"] = key
        _cache["prep"] = _host_prep(edge_index)
        _cache.pop("runner", None)
    dinv, idx_w, dsel_tab, bcap, TBg, cumb, WOFF = _cache["prep"]
    Ws, Bs = _fold_weights(inputs)
    bf = ml_dtypes.bfloat16

    x = np.asarray(inputs["x"], np.float32)
    xpad = np.zeros((NC, SH, IN), np.float32)
    xpad[:, :SR] = x.reshape(NC, SR, IN)
    dinvpad = np.ones((NC, SH), np.float32)
    dinvpad[:, :SR] = dinv.reshape(NC, SR)
    rdinvpad = 1.0 / dinvpad

    BWMAX = max(
        GROUPS[g] * int(bcap[g][qq]) for g in range(NG) for qq in range(NQ)
    )
    iotaw = np.tile(np.arange(P, dtype=np.float32)[None, :], (P, BWMAX))

    if "runner" not in _cache:
        nc = _build_nc(bcap, TBg, cumb, WOFF)
        _cache["runner"] = _SpmdRunner(nc, NC)
    r = _cache["runner"]

    in_maps = []
    for c in range(NC):
        m = {
            "xT": np.ascontiguousarray(xpad[c].T).astype(bf),
            "dinv": np.ascontiguousarray(
                dinvpad[c].reshape(TP, P).T
            ),
            "rdinv": rdinvpad[c].reshape(1, SH).astype(bf),
            "idx": idx_w[c],
            "dsel": dsel_tab[c],
            "iotaw": iotaw,
            "ident": np.eye(P, dtype=np.float32).astype(bf),
        }
        for i in range(5):
            m[f"W{i+1}"] = Ws[i].astype(bf)
            m[f"B{i+1}"] = Bs[i].astype(bf)
        in_maps.append(m)

    r.put_inputs(in_maps)
    outs = r.run()
    res = r.results(outs)
    y = np.concatenate([res[c]["y"][:SR] for c in range(NC)], axis=0)[:N]
    return np.ascontiguousarray(y, dtype=np.float32)
